# revision 1
# baseline (speedup 1.0000x reference)
"""Bidirectional-ALiBi bias kernel for Trainium2 (Bass/Tile), 8-core SPMD.

Computes out[h, i, j] = |j - i| * m where m = alpha[h] on the first
row/column, gamma[h] above the diagonal, beta[h] below it, and 0 on the
(non-edge) diagonal.  Output [16, 2048, 2048] f32, sharded 2 heads/core.

Strategy: every interior row i is a shifted window of a single per-head
profile vector V(k) = gamma*max(k,0) + beta*max(-k,0), k = j - i.  Each
core materializes one [128, 4095] SBUF tile W with W[p, c] = V(c-p-2047)
per head, then 16 plain rectangular DMAs per head stream the full
[2048, 2047] interior out of it (block t reads W[:, 2048-128t : ...]).
Row 0 / column 0 (alpha edges) are patched by two small DMAs from a
[2, 2048] tile R[h, j] = alpha[h]*j.  The kernel is pure DMA at
steady state: ~33.5 MB of HBM writes per core vs ~50 us of DVE work.
"""

import numpy as np

H = 16
S = 2048
P = 128
N_CORES = 8
H_LOC = H // N_CORES  # 2 heads per core
WID = 2 * S - 1  # profile width; index c in [0, WID), k = c - p - (S-1)
NT = S // P  # 16 row blocks per head

_NC = None


def _build():
    import concourse.bacc as bacc
    import concourse.mybir as mybir
    from concourse.tile import TileContext

    f32 = mybir.dt.float32
    nc = bacc.Bacc("TRN2", target_bir_lowering=False, debug=False)

    alpha_d = nc.dram_tensor("alpha", [H_LOC], f32, kind="ExternalInput").ap()
    beta_d = nc.dram_tensor("beta", [H_LOC], f32, kind="ExternalInput").ap()
    gamma_d = nc.dram_tensor("gamma", [H_LOC], f32, kind="ExternalInput").ap()
    out_d = nc.dram_tensor("out", [H_LOC, S, S], f32, kind="ExternalOutput").ap()

    with TileContext(nc) as tc:
        with (
            tc.tile_pool(name="const", bufs=1) as cpool,
            tc.tile_pool(name="work", bufs=2) as wpool,
        ):
            # K[p, c] = c - p - (S-1)  (the relative offset k = j - i)
            K = cpool.tile([P, WID], f32)
            nc.gpsimd.iota(
                K[:],
                pattern=[[1, WID]],
                base=-(S - 1),
                channel_multiplier=-1,
                allow_small_or_imprecise_dtypes=True,
            )

            # Edge-fix source: R[h, j] = alpha_h * j for j in [0, S)
            a2 = cpool.tile([H_LOC, 1], f32)
            nc.sync.dma_start(out=a2[:], in_=alpha_d[:])
            J2 = cpool.tile([H_LOC, S], f32)
            nc.gpsimd.iota(
                J2[:],
                pattern=[[1, S]],
                base=0,
                channel_multiplier=0,
                allow_small_or_imprecise_dtypes=True,
            )
            R = cpool.tile([H_LOC, S], f32)
            nc.vector.tensor_scalar_mul(R[:], J2[:], a2[:])

            for h in range(H_LOC):
                # broadcast per-head coefficients to [P, 1]
                gb = wpool.tile([P, 1], f32, tag="gb")
                nc.sync.dma_start(out=gb[:], in_=gamma_d[h : h + 1].to_broadcast((P, 1)))
                bb = wpool.tile([P, 1], f32, tag="bb")
                nc.sync.dma_start(out=bb[:], in_=beta_d[h : h + 1].to_broadcast((P, 1)))
                nbb = wpool.tile([P, 1], f32, tag="nbb")
                nc.vector.tensor_scalar_mul(nbb[:], bb[:], -1.0)

                # W[p, c] = gamma*max(k,0) + beta*max(-k,0),  k = c - p - (S-1)
                T1 = wpool.tile([P, WID], f32, tag="T1")
                nc.vector.tensor_scalar(
                    out=T1[:],
                    in0=K[:],
                    scalar1=0.0,
                    scalar2=gb[:],
                    op0=mybir.AluOpType.max,
                    op1=mybir.AluOpType.mult,
                )
                T2 = wpool.tile([P, WID], f32, tag="T2")
                nc.vector.tensor_scalar(
                    out=T2[:],
                    in0=K[:],
                    scalar1=0.0,
                    scalar2=nbb[:],
                    op0=mybir.AluOpType.min,
                    op1=mybir.AluOpType.mult,
                )
                W = wpool.tile([P, WID], f32, tag="W")
                nc.vector.tensor_add(out=W[:], in0=T1[:], in1=T2[:])

                # Interior: rows >= 1, cols >= 1.
                # Block t covers rows i = P*t + p; col j reads W[p, j + S-1 - P*t].
                o = out_d[h]
                nc.sync.dma_start(out=o[1:P, 1:S], in_=W[1:P, S:WID])
                for t in range(1, NT):
                    nc.sync.dma_start(
                        out=o[P * t : P * (t + 1), 1:S],
                        in_=W[:, S - P * t : WID - P * t],
                    )

            # Edge fixes (both heads in one DMA each):
            # col 0: out[h, i, 0] = alpha_h * i   (covers (0,0) = 0)
            with nc.allow_non_contiguous_dma(reason="column-0 edge fix, 4B/row"):
                nc.sync.dma_start(out=out_d[:, :, 0:1], in_=R[:, :])
            # row 0: out[h, 0, 1:S] = alpha_h * j
            nc.sync.dma_start(out=out_d[:, 0:1, 1:S], in_=R[:, 1:S])

    nc.compile()
    return nc


def _run(alpha, beta, gamma, **spmd_kwargs):
    """Compile (cached) and run on the 8 NeuronCores; returns BassKernelResults."""
    global _NC
    if _NC is None:
        _NC = _build()
    from concourse import bass_utils

    alpha = np.ascontiguousarray(alpha, dtype=np.float32)
    beta = np.ascontiguousarray(beta, dtype=np.float32)
    gamma = np.ascontiguousarray(gamma, dtype=np.float32)
    in_maps = [
        {
            "alpha": alpha[c * H_LOC : (c + 1) * H_LOC],
            "beta": beta[c * H_LOC : (c + 1) * H_LOC],
            "gamma": gamma[c * H_LOC : (c + 1) * H_LOC],
        }
        for c in range(N_CORES)
    ]
    return bass_utils.run_bass_kernel_spmd(
        _NC, in_maps, core_ids=list(range(N_CORES)), **spmd_kwargs
    )


def kernel(alpha, beta, gamma, seq_len):
    assert int(seq_len) == S, f"kernel hardcodes seq_len={S}, got {seq_len}"
    res = _run(alpha, beta, gamma)
    return np.concatenate([r["out"] for r in res.results], axis=0)


# revision 3
# speedup vs baseline: 1.7273x; 1.7273x over previous
"""Bidirectional-ALiBi bias kernel for Trainium2 (Bass/Tile), 8-core SPMD.

Computes out[h, i, j] = |j - i| * m where m = alpha[h] on the first
row/column, gamma[h] above the diagonal, beta[h] below it, and 0 on the
(non-edge) diagonal.  Output [16, 2048, 2048] f32, sharded 2 heads/core.

Strategy: every interior row i is a shifted window of a per-head profile
vector V(k) = gamma*max(k,0) + beta*max(-k,0), k = j - i.  Each core
materializes a diagonalized SBUF image W[p, c] = V(c - p - (S-1)) per
head (in column chunks, for pipelining); then plain rectangular DMAs
stream the [2048, 2047] interior out of it — row-block t of the output
is W[:, S-128t : ...].  Row 0 (alpha edge) is folded into a patched
copy W2 used by block 0; column 0 is a 4B-scatter DMA from a
partition-spread tile R[p, t] = alpha*(128t+p), issued on the gpsimd
SWDGE ring to keep the two HWDGE rings free for interior streaming.

Hardware notes (from NTFF profiling): each of the 16 SDMA engines tops
out at ~26.5 GB/s, and a DMA whose outer row count is not a multiple of
16 lands on a single engine — so every bulk DMA here is 128 rows.
Interior DMAs alternate between the SP and Activation HWDGE rings.
"""

import numpy as np

H = 16
S = 2048
P = 128
N_CORES = 8
H_LOC = H // N_CORES  # 2 heads per core
WID = 2 * S - 1  # profile width; index c in [0, WID), k = c - p - (S-1)
NT = S // P  # 16 row blocks per head
NCH = 4  # W column chunks
CW = (WID + NCH - 1) // NCH  # 1024

_NC = None


def _build():
    import concourse.bacc as bacc
    import concourse.mybir as mybir
    from concourse.tile import TileContext

    f32 = mybir.dt.float32
    nc = bacc.Bacc("TRN2", target_bir_lowering=False, debug=False)

    alpha_d = nc.dram_tensor("alpha", [H_LOC], f32, kind="ExternalInput").ap()
    beta_d = nc.dram_tensor("beta", [H_LOC], f32, kind="ExternalInput").ap()
    gamma_d = nc.dram_tensor("gamma", [H_LOC], f32, kind="ExternalInput").ap()
    out_d = nc.dram_tensor("out", [H_LOC, S, S], f32, kind="ExternalOutput").ap()

    # chunk ci covers c in [lo, hi)
    bounds = [(ci * CW, min((ci + 1) * CW, WID)) for ci in range(NCH)]
    # emit order: high chunks first (they serve block 0 / low t, incl. W2)
    order = list(range(NCH - 1, -1, -1))

    hw_rings = None  # set inside build
    ring_i = 0

    with TileContext(nc) as tc:
        hw_rings = [nc.sync, nc.scalar]
        with (
            tc.tile_pool(name="coef", bufs=1) as cpool,
            tc.tile_pool(name="kpool", bufs=NCH) as kpool,
            tc.tile_pool(name="wpool", bufs=NCH * H_LOC) as wpool,
            tc.tile_pool(name="w2pool", bufs=H_LOC) as w2pool,
            tc.tile_pool(name="tpool", bufs=3) as tpool,
        ):
            # per-head coefficients broadcast to all partitions: [128, 2]
            A2 = cpool.tile([P, H_LOC], f32)
            nc.scalar.dma_start(out=A2[:], in_=alpha_d.partition_broadcast(P))
            B2 = cpool.tile([P, H_LOC], f32)
            nc.scalar.dma_start(out=B2[:], in_=beta_d.partition_broadcast(P))
            G2 = cpool.tile([P, H_LOC], f32)
            nc.scalar.dma_start(out=G2[:], in_=gamma_d.partition_broadcast(P))
            NB2 = cpool.tile([P, H_LOC], f32)
            nc.vector.tensor_scalar_mul(NB2[:], B2[:], -1.0)

            # column-0 fix: R[p, t] = alpha_h * (128t + p), DMAed as a 4B
            # scatter down column 0 (p-major traversal so descriptors
            # spread across all SBUF ports / SDMA engines).
            IB = cpool.tile([P, NT], f32)
            nc.gpsimd.iota(
                IB[:],
                pattern=[[P, NT]],
                base=0,
                channel_multiplier=1,
                allow_small_or_imprecise_dtypes=True,
            )
            for h in range(H_LOC):
                Rh = cpool.tile([P, NT], f32, tag=f"R{h}")
                nc.vector.tensor_scalar_mul(Rh[:], IB[:], A2[:, h : h + 1])
                col_out = out_d[h, :, 0:1].rearrange("(t p) o -> p t o", p=P)
                with nc.allow_non_contiguous_dma(reason="column-0 edge fix"):
                    nc.gpsimd.dma_start(out=col_out, in_=Rh[:])

            Ks = {}
            Ws = {}
            for ci in order:
                lo, hi = bounds[ci]
                w = hi - lo
                # K[p, c-lo] = c - p - (S-1)
                Kc = kpool.tile([P, CW], f32, tag="K")
                nc.gpsimd.iota(
                    Kc[:, :w],
                    pattern=[[1, w]],
                    base=lo - (S - 1),
                    channel_multiplier=-1,
                    allow_small_or_imprecise_dtypes=True,
                )
                Ks[ci] = Kc
                for h in range(H_LOC):
                    T1 = tpool.tile([P, CW], f32, tag="T1")
                    nc.vector.tensor_scalar(
                        out=T1[:, :w],
                        in0=Kc[:, :w],
                        scalar1=G2[:, h : h + 1],
                        scalar2=0.0,
                        op0=mybir.AluOpType.mult,
                        op1=mybir.AluOpType.max,
                    )
                    T2 = tpool.tile([P, CW], f32, tag="T2")
                    nc.vector.tensor_scalar(
                        out=T2[:, :w],
                        in0=Kc[:, :w],
                        scalar1=NB2[:, h : h + 1],
                        scalar2=0.0,
                        op0=mybir.AluOpType.mult,
                        op1=mybir.AluOpType.max,
                    )
                    Wc = wpool.tile([P, CW], f32, tag="W")
                    nc.vector.tensor_add(out=Wc[:, :w], in0=T1[:, :w], in1=T2[:, :w])
                    Ws[(h, ci)] = Wc

                    # interior sub-DMAs for row blocks t >= 1 within this chunk:
                    # block t, col j reads c = j + S-1-128t, j in [1, S)
                    for t in range(1, NT):
                        c_lo = max(S - P * t, lo)
                        c_hi = min(WID - P * t, hi)
                        if c_lo >= c_hi:
                            continue
                        j_lo = c_lo - (S - 1 - P * t)
                        j_hi = c_hi - (S - 1 - P * t)
                        ring = hw_rings[ring_i % 2]
                        ring_i += 1
                        ring.dma_start(
                            out=out_d[h, P * t : P * (t + 1), j_lo:j_hi],
                            in_=Wc[:, c_lo - lo : c_hi - lo],
                        )

                # after the top two chunks of a head exist, build W2 for
                # block 0: rows 1..127 are W[1:, S:WID]; row 0 is alpha*j.
                if ci == order[1]:  # chunks NCH-1 and NCH-2 are ready
                    for h in range(H_LOC):
                        W2 = w2pool.tile([P, S - 1], f32, tag="W2")
                        # c in [S, WID) spans chunks 2 and 3 (CW=1024, S=2CW)
                        cA, cB = NCH - 2, NCH - 1
                        loA, hiA = bounds[cA]
                        loB, hiB = bounds[cB]
                        wA = hiA - loA  # 1024; covers c in [2048, 3072)
                        wB = hiB - loB  # 1023; covers c in [3072, 4095)
                        nc.vector.tensor_copy(
                            out=W2[:, 0:wA], in_=Ws[(h, cA)][:, :wA]
                        )
                        nc.vector.tensor_copy(
                            out=W2[:, wA : wA + wB], in_=Ws[(h, cB)][:, :wB]
                        )
                        # row 0: alpha_h * j ; K rows at p=0 hold exactly j
                        nc.vector.tensor_scalar_mul(
                            W2[0:1, 0:wA], Ks[cA][0:1, :wA], A2[0:1, h : h + 1]
                        )
                        nc.vector.tensor_scalar_mul(
                            W2[0:1, wA : wA + wB],
                            Ks[cB][0:1, :wB],
                            A2[0:1, h : h + 1],
                        )
                        ring = hw_rings[ring_i % 2]
                        ring_i += 1
                        ring.dma_start(out=out_d[h, 0:P, 1:S], in_=W2[:])

    nc.compile()
    return nc


def _run(alpha, beta, gamma, **spmd_kwargs):
    """Compile (cached) and run on the 8 NeuronCores; returns BassKernelResults."""
    global _NC
    if _NC is None:
        _NC = _build()
    from concourse import bass_utils

    alpha = np.ascontiguousarray(alpha, dtype=np.float32)
    beta = np.ascontiguousarray(beta, dtype=np.float32)
    gamma = np.ascontiguousarray(gamma, dtype=np.float32)
    in_maps = [
        {
            "alpha": alpha[c * H_LOC : (c + 1) * H_LOC],
            "beta": beta[c * H_LOC : (c + 1) * H_LOC],
            "gamma": gamma[c * H_LOC : (c + 1) * H_LOC],
        }
        for c in range(N_CORES)
    ]
    return bass_utils.run_bass_kernel_spmd(
        _NC, in_maps, core_ids=list(range(N_CORES)), **spmd_kwargs
    )


def kernel(alpha, beta, gamma, seq_len):
    assert int(seq_len) == S, f"kernel hardcodes seq_len={S}, got {seq_len}"
    res = _run(alpha, beta, gamma)
    return np.concatenate([r["out"] for r in res.results], axis=0)
